# revision 48
# baseline (speedup 1.0000x reference)
"""MeshGraphNet processor on 8 Trainium2 NeuronCores.

Strategy (edge-cut graph partition):
  - Nodes dealt round-robin by in-degree rank to 8 cores (1250 each, padded
    to 1280 slots/core).  Each core owns all edges whose dst is local, so
    the segment-sum is core-local.  Per-rank-position degrees are padded to
    the max across cores so one SPMD program (fixed APs) serves every core.
  - Per layer: the local node shard is transposed, cast bf16, and
    AllGather'd into a row-major DRAM table (the collective is fully hidden
    behind compute on HW).  Only n[src] is gathered -- dma_gather transpose
    mode, 512-idx calls spread over 4 SWDGE queues (Q7 descriptor
    generation is the dominant cost and parallelizes across queues).
    n[dst] needs no gather: with edges sorted by dst it is piecewise
    constant, so the per-node dst term ydst = ew0d.T @ n is broadcast to
    edge columns with zero-stride APs and injected into PSUM through an
    identity matmul.  Pad edges would pick up the node's dst term, so a
    per-node "phantom edge" (src=0 input) is run through the same edge MLP
    and (pad count) * phantom is subtracted from the segment sum.
  - Edge MLP runs fp16 end to end (fp16 residual carrier, fp16 h1/h2, raw
    fp16 weights) in a 3-stage software pipeline over 1024-col blocks so
    the PE never stalls on the ACT relus; matmul outputs split per 512-col
    PSUM bank.  Segment-sum is strided DVE reduces by degree class (fp16
    agg); the node MLP keeps an fp32 residual with an fp16 matmul shadow.
  - Wire diet: edge features, weights, and the output ship fp16; weights
    upload 1/8 per core and AllGather on device.  The runner stages inputs
    with per-device puts overlapped with the AOT NEFF compile; the timed
    loop measures dispatch+execute+sync (output download excluded -- its
    tunnel jitter would swamp the differential timing used by test.py).
"""

import sys
import threading
import time
import numpy as np

import jax
import concourse.tile as tile
from concourse import bacc, mybir
from concourse.masks import make_identity


def _warm_isa():
    # get_isa is functools.cache'd: the expensive cffi/pycparser ISA parse
    # (~1.4s) can run while the caller loads inputs and host prep proceeds
    from concourse.isa import get_isa
    get_isa("TRN2")


_ISA_WARM = threading.Thread(target=_warm_isa, daemon=True)
_ISA_WARM.start()

P = 15
D = 128
CORES = 8
SH = 1250          # real nodes per core
SHP = 1280         # padded slots per core (multiple of 128)
NT = CORES * SHP   # 10240 table slots
BLK = 1024         # edge MLP block (2 PSUM banks)
PSP_BUFS = 3       # PSUM pool depth (x2 banks each)
BP_BUFS = 6        # h1/h2 pool depth
GP_BUFS = 18       # gather pool depth
LAST_EXEC_S = None # wall time of the device dispatch+run, set per call
GCH = 512          # edges per dma_gather call
NSWQ = 4           # SWDGE queues (1-4)
QSPREAD = True     # spread gather calls across SWDGE queues
FILL_POOL = False   # all ydst_e fill classes on gpsimd (else alternate)

# packed weight layout: 9 fp16 [128,128] blocks per layer
#   0=ew0s 1=ew0d 2=ew0e 3=ew1 4=ew2 5=nw0a 6=nw0b 7=nw1 8=nw2
# block 135 = biases: eb0@0 eb1@15 eb2@30 nb0@45 nb1@60 nb2@75 (cols, 15 each)
# the pack is sharded 1/8 per core on the wire and AllGather'd on device:
# core c uploads blocks [c*WPC, (c+1)*WPC)
WBLKS = 136
WPC = WBLKS // CORES

F32 = mybir.dt.float32
F16 = mybir.dt.float16
BF16 = mybir.dt.bfloat16
I16 = mybir.dt.int16
AF = mybir.ActivationFunctionType

_T = {}
_CACHE = {}


def rerun_reps(reps, iters=10, ablate=()):
    """Re-run the staged problem with the whole network unrolled `reps`
    times inside one NEFF (same I/O).  Used by test.py for differential
    device timing: (T_reps - T_1)/(reps-1) cancels the tunnel RTT and
    the output-download time, leaving pure device execution per pass.
    Requires a prior kernel() call in this process (uses its staged
    device inputs)."""
    c = _CACHE
    nc = _build_kernel(c["cap"], c["classes"], c["seg_start"], reps=reps,
                       ablate=ablate)
    _, exec_s = _run_spmd_timed(nc, c["in_maps"], staged=c["staged"],
                                iters=iters)
    return exec_s


def _tic(name, t0):
    t = time.time()
    _T[name] = _T.get(name, 0.0) + (t - t0)
    return t


def _slot(core, local):
    # slot id chosen so that byte offset in the AllGather output equals
    # slot*256: out row = 128*core + (local % 128), col block = local // 128
    c, p = divmod(int(local), 128)
    return (128 * int(core) + p) * (SHP // 128) + c


def _partition_graph(src, dst, n_nodes):
    indeg = np.bincount(dst, minlength=n_nodes)
    order = np.argsort(-indeg, kind="stable")  # degree desc
    core_nodes = [[] for _ in range(CORES)]
    for r, v in enumerate(order):
        core_nodes[r % CORES].append(v)
    # local order: degree ascending (reverse of dealt order)
    core_nodes = [list(reversed(cn)) for cn in core_nodes]
    assert all(len(cn) == SH for cn in core_nodes)
    # padded degree per rank position (same for every core)
    degs = np.stack([indeg[cn] for cn in core_nodes])  # [CORES, SH]
    dpos = degs.max(axis=0)                            # [SH] ascending-ish
    return core_nodes, degs, dpos


def _degree_classes(dpos):
    """Contiguous position ranges with equal padded degree (skip d==0)."""
    classes = []
    a = 0
    while a < SH:
        b = a
        while b < SH and dpos[b] == dpos[a]:
            b += 1
        if dpos[a] > 0:
            classes.append((int(dpos[a]), a, b))
        a = b
    return classes


def _build_kernel(cap, classes, seg_start, reps=1, ablate=()):
    nc = bacc.Bacc("TRN2", target_bir_lowering=False, debug=False,
                   num_devices=CORES, num_swdge_queues=NSWQ)
    t_ef = nc.dram_tensor("ef", [D, cap], F16, kind="ExternalInput")
    t_nf = nc.dram_tensor("nf", [D, SHP], F32, kind="ExternalInput")
    t_sxs = nc.dram_tensor("sidxs", [16, cap // 16], I16,
                           kind="ExternalInput")
    t_cnt = nc.dram_tensor("cnt", [128, SHP], F16, kind="ExternalInput")
    t_wpk = nc.dram_tensor("wpk", [128, WPC * 128], F16,
                           kind="ExternalInput")
    t_out = nc.dram_tensor("outn", [D, SH], F16, kind="ExternalOutput")
    # tiny completion beacon: written after the final store, fetched inside
    # the timed loop (a full-output download there would add tunnel jitter;
    # block_until_ready alone acks at dispatch, not completion)
    t_tick = nc.dram_tensor("tick", [1, 16], F16, kind="ExternalOutput")

    nblk = cap // BLK

    with tile.TileContext(nc) as tc:
        with (
            tc.tile_pool(name="persist", bufs=1) as pp,
            tc.tile_pool(name="wts", bufs=2) as wp,
            tc.tile_pool(name="nsh", bufs=2) as np_,
            tc.tile_pool(name="aggp", bufs=2) as ap_,
            tc.tile_pool(name="gath", bufs=GP_BUFS) as gp,
            tc.tile_pool(name="blk", bufs=BP_BUFS) as bp,
            tc.tile_pool(name="stg", bufs=2) as sp,
            tc.tile_pool(name="dgath", bufs=1) as dgp,
            tc.tile_pool(name="ps", bufs=PSP_BUFS, space="PSUM") as psp,
            tc.tile_pool(name="pst", bufs=2, space="PSUM") as pstp,
            tc.tile_pool(name="dram", bufs=2, space="DRAM") as dp,
        ):
            ident = pp.tile([128, 128], F32)
            make_identity(nc, ident[:])
            identF = pp.tile([128, 128], F16, name="identF")
            nc.scalar.activation(identF[:], ident[:], AF.Copy)
            # gather indices ship once (16 partitions) and are replicated
            # into the 8 Q7 groups on device
            sidxs = pp.tile([128, cap // 16], I16, name="sidxs")
            for g in range(8):
                nc.sync.dma_start(sidxs[16 * g:16 * (g + 1), :], t_sxs.ap())
            # per-node pad-edge counts (phantom-edge correction)
            cnt = pp.tile([128, SHP], F16, name="cnt")
            nc.sync.dma_start(cnt[:], t_cnt.ap())
            # phantom pad-edge feature carrier, reset each pass
            pv = pp.tile([D, SHP], F16, name="pv")

            # ---- weight pack arrives 1/8 per core; AllGather on device
            #      (same SBUF->DRAM staging idiom as the node table) ----
            wstg = pp.tile([128, WPC * 128], F16, name="wstg")
            nc.sync.dma_start(wstg[:], t_wpk.ap())
            ag_win = dp.tile([128, WPC * 128], F16, tag="agwi")
            nc.sync.dma_start(ag_win[:], wstg[:])
            ag_wout = dp.tile([128 * CORES, WPC * 128], F16, tag="agwo",
                              addr_space="Shared")
            nc.gpsimd.collective_compute(
                "AllGather", mybir.AluOpType.bypass,
                replica_groups=[list(range(CORES))],
                ins=[ag_win[:].opt()], outs=[ag_wout[:].opt()])

            def wblk(b):
                c, i = divmod(b, WPC)
                return ag_wout[128 * c:128 * (c + 1),
                               128 * i:128 * (i + 1)]

            bc, bi = divmod(WBLKS - 1, WPC)
            b16 = pp.tile([128, 96], F16, name="b16")
            nc.sync.dma_start(
                b16[:, 0:96],
                ag_wout[128 * bc:128 * (bc + 1), 128 * bi:128 * bi + 96])
            bias = pp.tile([128, 96], F32, name="bias")
            nc.scalar.activation(bias[:], b16[:], AF.Copy)
            BOF = dict(eb0=0, eb1=15, eb2=30, nb0=45, nb1=60, nb2=75)

            def bslc(k, l):
                return bias[:, BOF[k] + l:BOF[k] + l + 1]


            # fp16 edge-feature carrier: loaded straight from the wire
            # buffer, residuals accumulate in fp16 (10-bit mantissa)
            e = pp.tile([D, cap], F16, name="e")
            for rep in range(reps):
              # ---- per-pass reload: node shard + edge features ----
              n_cur = np_.tile([D, SHP], F32, tag="n")
              nc.sync.dma_start(n_cur[:], t_nf.ap())
              nc.sync.dma_start(e[:], t_ef.ap())
              nc.gpsimd.memset(pv[:], 0.0)

              for l in range(P):
                # ---- this layer's weights: 9 fp16 block DMAs; only the
                #      src/dst gather blocks (bf16 to match the table) and
                #      nw0a (f32r against the fp32 n_cur) need conversion;
                #      everything else is used as raw fp16 ----
                w16 = wp.tile([128, 9 * 128], F16, tag="w16")
                for i in range(9):
                    nc.sync.dma_start(w16[:, 128 * i:128 * (i + 1)],
                                      wblk(l * 9 + i))
                wsd = wp.tile([128, 128], BF16, tag="wsd")
                nc.scalar.activation(wsd[:], w16[:, 0:128], AF.Copy)
                # fp16 shadow of the node features for the node-MLP matmul
                # (the fp32 carrier stays for the residual)
                n16 = wp.tile([D, SHP], F16, tag="n16")
                nc.scalar.activation(n16[:], n_cur[:], AF.Copy)
                # dst contribution per NODE: ydst = ew0d.T @ n16.  dst-sorted
                # edges make n[dst] piecewise-constant, so the per-edge dst
                # term is a zero-stride broadcast of ydst -- no gather.
                ydst = wp.tile([D, SHP], F16, tag="ydst")
                for (s0, w_) in ((0, 512), (512, 512), (1024, 256)):
                    psd = psp.tile([128, BLK], F32, tag="ps")
                    nc.tensor.matmul(psd[:, :w_], w16[:, 128:256],
                                     n16[:, s0:s0 + w_], start=True,
                                     stop=True)
                    nc.scalar.activation(ydst[:, s0:s0 + w_], psd[:, :w_],
                                         AF.Copy)

                # ---- publish n_cur: transpose shard to rows, cast bf16,
                #      AllGather into the row-major node table ----
                stage = sp.tile([128, SHP], BF16, tag="stage")
                for c in range(SHP // 128):
                    pt = pstp.tile([128, 128], F32, tag="pt")
                    nc.tensor.transpose(
                        pt[:], n_cur[:, c * 128:(c + 1) * 128], ident[:])
                    nc.scalar.activation(
                        stage[:, c * 128:(c + 1) * 128], pt[:], AF.Copy)
                ag_in = dp.tile([128, SHP], BF16, tag="agin")
                nc.sync.dma_start(ag_in[:], stage[:])
                ag_out = dp.tile([128 * CORES, SHP], BF16, tag="agout",
                                 addr_space="Shared")
                # expand ydst to edge columns (broadcast per degree
                # class); pad columns get the node's value too -- corrected
                # via the phantom-edge pass below
                ydst_e = dgp.tile([128, cap], F16, tag="ydste")
                need = int(seg_start[SH])
                if cap > need:
                    nc.gpsimd.memset(ydst_e[:, need:cap], 0.0)
                for ci, (d, a, b) in enumerate(classes):
                    s = seg_start[a]
                    dst_ap = ydst[:, a:b].rearrange(
                        "p (n o) -> p n o", o=1).broadcast_to((128, b - a, d))
                    out_ap = ydst_e[:, s:s + (b - a) * d].rearrange(
                        "p (n d) -> p n d", d=d)
                    eng = nc.gpsimd if (FILL_POOL or ci % 2 == 0) else nc.vector
                    eng.tensor_copy(out_ap, dst_ap)
                if "nocoll" in ablate:
                    nc.sync.dma_start(ag_out[0:128, :], ag_in[:])
                else:
                    nc.gpsimd.collective_compute(
                        "AllGather", mybir.AluOpType.bypass,
                        replica_groups=[list(range(CORES))],
                        ins=[ag_in[:].opt()], outs=[ag_out[:].opt()])
                table_ap = ag_out[:].rearrange("r (c f) -> (r c) f", f=D)

                # ---- edge MLP blocks, 3-stage software pipeline: the PE
                #      queue sees mm1(j), mm2(j-1), mm3(j-2) per iteration
                #      so it never stalls on the ACT relu of its own block
                # a matmul output must stay inside one 512-col PSUM bank:
                # emit per-bank halves (ACT/DVE still see 1024-wide tiles)
                def mm(ps, lhsT, rhs, start, stop):
                    for h in range(BLK // 512):
                        hs = slice(h * 512, (h + 1) * 512)
                        nc.tensor.matmul(ps[:, hs], lhsT, rhs[:, hs],
                                         start=start, stop=stop)

                st_h1 = {}
                st_h2 = {}
                for j in range(nblk + 2):
                    if j < nblk:
                        gs = gp.tile([128, 1, BLK], BF16, tag="gs")
                        for k in range(BLK // GCH):
                            if "srcg_half" in ablate and k % 2 == 1:
                                continue
                            g = j * (BLK // GCH) + k
                            isl = slice(g * (GCH // 16),
                                        (g + 1) * (GCH // 16))
                            nc.gpsimd.dma_gather(
                                gs[:, :, k * GCH:(k + 1) * GCH], table_ap,
                                sidxs[:, isl], GCH, GCH, D, transpose=True,
                                queue_num=(g % NSWQ) if QSPREAD else 0,
                            )
                        eblk = e[:, j * BLK:(j + 1) * BLK]
                        ps1 = psp.tile([128, BLK], F32, tag="ps")
                        mm(ps1, wsd[:, 0:128], gs[:, 0, :],
                           start=True, stop=False)
                        mm(ps1, identF[:],
                           ydst_e[:, j * BLK:(j + 1) * BLK],
                           start=False, stop=False)
                        mm(ps1, w16[:, 256:384], eblk,
                           start=False, stop=True)
                        h1 = bp.tile([D, BLK], F16, tag="h1")
                        nc.scalar.activation(h1[:], ps1[:], AF.Relu,
                                             bias=bslc("eb0", l))
                        st_h1[j] = h1
                    if j >= 1 and j - 1 < nblk:
                        ps2 = psp.tile([128, BLK], F32, tag="ps")
                        mm(ps2, w16[:, 384:512], st_h1.pop(j - 1)[:],
                           start=True, stop=True)
                        h2 = bp.tile([D, BLK], F16, tag="h2")
                        nc.scalar.activation(h2[:], ps2[:], AF.Relu,
                                             bias=bslc("eb1", l))
                        st_h2[j - 1] = h2
                    if j >= 2:
                        jj = j - 2
                        ps3 = psp.tile([128, BLK], F32, tag="ps")
                        mm(ps3, w16[:, 512:640], st_h2.pop(jj)[:],
                           start=True, stop=True)
                        ebj = e[:, jj * BLK:(jj + 1) * BLK]
                        # e += ps3 + eb2, fused (DVE: gpsimd can't read PSUM)
                        nc.vector.scalar_tensor_tensor(
                            out=ebj, in0=ps3[:], scalar=bslc("eb2", l),
                            in1=ebj, op0=mybir.AluOpType.add,
                            op1=mybir.AluOpType.add)

                # ---- phantom pad-edge pass: pad columns run the edge
                # MLP with src=0 and the node's own dst term, exactly like
                # the real pads embedded in e, so (pad count) * pv can be
                # subtracted from the segment sum afterwards ----
                for (s0, w_) in ((0, 512), (512, 512), (1024, 256)):
                    psA = psp.tile([128, BLK], F32, tag="ps")
                    nc.tensor.matmul(psA[:, :w_], identF[:],
                                     ydst[:, s0:s0 + w_], start=True,
                                     stop=False)
                    nc.tensor.matmul(psA[:, :w_], w16[:, 256:384],
                                     pv[:, s0:s0 + w_], start=False,
                                     stop=True)
                    h1p = bp.tile([D, BLK], F16, tag="h1")
                    nc.scalar.activation(h1p[:, :w_], psA[:, :w_], AF.Relu,
                                         bias=bslc("eb0", l))
                    psB = psp.tile([128, BLK], F32, tag="ps")
                    nc.tensor.matmul(psB[:, :w_], w16[:, 384:512],
                                     h1p[:, :w_], start=True, stop=True)
                    h2p = bp.tile([D, BLK], F16, tag="h2")
                    nc.scalar.activation(h2p[:, :w_], psB[:, :w_], AF.Relu,
                                         bias=bslc("eb1", l))
                    psC = psp.tile([128, BLK], F32, tag="ps")
                    nc.tensor.matmul(psC[:, :w_], w16[:, 512:640],
                                     h2p[:, :w_], start=True, stop=True)
                    nc.vector.scalar_tensor_tensor(
                        out=pv[:, s0:s0 + w_], in0=psC[:, :w_],
                        scalar=bslc("eb2", l), in1=pv[:, s0:s0 + w_],
                        op0=mybir.AluOpType.add, op1=mybir.AluOpType.add)

                # ---- segment sum (edges sorted by dst, degree classes);
                #      fp16 in/out rides the DVE 2x 16-bit mode ----
                agg = ap_.tile([D, SHP], F16, tag="agg")
                nc.gpsimd.memset(agg[:], 0.0)
                with nc.allow_low_precision("fp16 agg, values O(10)"):
                    for (d, a, b) in classes:
                        s = seg_start[a]
                        seg = e[:, s:s + (b - a) * d].rearrange(
                            "p (n d) -> p n d", d=d)
                        nc.vector.tensor_reduce(
                            agg[:, a:b], seg, axis=mybir.AxisListType.X,
                            op=mybir.AluOpType.add)
                    # subtract the pad-edge pollution: agg -= cnt * pv
                    tmpc = sp.tile([D, SHP], F16, tag="tmpc")
                    nc.gpsimd.tensor_tensor(tmpc[:], pv[:], cnt[:],
                                            op=mybir.AluOpType.mult)
                    nc.gpsimd.tensor_tensor(agg[:], agg[:], tmpc[:],
                                            op=mybir.AluOpType.subtract)

                # ---- node MLP on local shard ----
                n_new = np_.tile([D, SHP], F32, tag="n")
                for (s0, w_) in ((0, 512), (512, 512), (1024, 256)):
                    ps1 = psp.tile([128, BLK], F32, tag="ps")
                    nc.tensor.matmul(ps1[:, :w_], w16[:, 640:768],
                                     n16[:, s0:s0 + w_], start=True,
                                     stop=False)
                    nc.tensor.matmul(ps1[:, :w_], w16[:, 768:896],
                                     agg[:, s0:s0 + w_], start=False,
                                     stop=True)
                    h1 = bp.tile([D, BLK], F16, tag="h1")
                    nc.scalar.activation(h1[:, :w_], ps1[:, :w_], AF.Relu,
                                         bias=bslc("nb0", l))
                    ps2 = psp.tile([128, BLK], F32, tag="ps")
                    nc.tensor.matmul(ps2[:, :w_], w16[:, 896:1024],
                                     h1[:, :w_], start=True, stop=True)
                    h2 = bp.tile([D, BLK], F16, tag="h2")
                    nc.scalar.activation(h2[:, :w_], ps2[:, :w_], AF.Relu,
                                         bias=bslc("nb1", l))
                    ps3 = psp.tile([128, BLK], F32, tag="ps")
                    nc.tensor.matmul(ps3[:, :w_], w16[:, 1024:1152],
                                     h2[:, :w_], start=True, stop=True)
                    nc.vector.scalar_tensor_tensor(
                        out=n_new[:, s0:s0 + w_], in0=ps3[:, :w_],
                        scalar=bslc("nb2", l), in1=n_cur[:, s0:s0 + w_],
                        op0=mybir.AluOpType.add, op1=mybir.AluOpType.add)

                if l == P - 1:
                    o16 = sp.tile([D, SHP], F16, tag="o16")
                    nc.scalar.activation(o16[:, :SH], n_new[:, :SH], AF.Copy)
                    nc.sync.dma_start(t_out.ap(), o16[:, :SH])
                    nc.sync.dma_start(t_tick.ap(), o16[0:1, 0:16])
                n_cur = n_new

    nc.compile()
    return nc


def _mesh_and_put():
    """(mesh, sharding, devices, put_global) for the 8-core mesh."""
    from jax.sharding import Mesh, PartitionSpec, NamedSharding

    devices = jax.devices()[:CORES]
    mesh = Mesh(np.asarray(devices), ("core",))
    ns = NamedSharding(mesh, PartitionSpec("core"))

    def put_global(arrs):
        shards = [jax.device_put(np.asarray(arrs[c]), devices[c])
                  for c in range(CORES)]
        shape = (CORES * shards[0].shape[0], *shards[0].shape[1:])
        return jax.make_array_from_single_device_arrays(shape, ns, shards)

    return mesh, ns, devices, put_global


def _run_spmd_timed(nc, in_maps, staged=None, iters=10):
    """run_bass_kernel_spmd replacement: async device_put (pre-staged where
    possible) overlapped with AOT NEFF compile; only
    dispatch+execute+output-fetch is timed."""
    from jax.sharding import PartitionSpec
    from jax.experimental.shard_map import shard_map
    from concourse.bass2jax import (_bass_exec_p, install_neuronx_cc_hook,
                                    partition_id_tensor)

    install_neuronx_cc_hook()
    n_cores = len(in_maps)
    partition_name = (nc.partition_id_tensor.name
                      if nc.partition_id_tensor else None)

    in_names, out_names, out_avals, zero_outs = [], [], [], []
    for alloc in nc.m.functions[0].allocations:
        if not isinstance(alloc, mybir.MemoryLocationSet):
            continue
        name = alloc.memorylocations[0].name
        if alloc.kind == "ExternalInput":
            if name != partition_name:
                in_names.append(name)
        elif alloc.kind == "ExternalOutput":
            shape = tuple(alloc.tensor_shape)
            dtype = mybir.dt.np(alloc.dtype)
            out_names.append(name)
            out_avals.append(jax.core.ShapedArray(shape, dtype))
            zero_outs.append(np.zeros(shape, dtype))
    n_params = len(in_names)
    n_outs = len(out_avals)
    all_in_names = list(in_names) + list(out_names)
    if partition_name is not None:
        all_in_names.append(partition_name)

    def _body(*args):
        operands = list(args)
        if partition_name is not None:
            operands.append(partition_id_tensor())
        outs = _bass_exec_p.bind(
            *operands,
            out_avals=tuple(out_avals),
            in_names=tuple(all_in_names),
            out_names=tuple(out_names),
            lowering_input_output_aliases=(),
            sim_require_finite=True,
            sim_require_nnan=True,
            nc=nc,
        )
        return tuple(outs)

    mesh, ns, _, put_global = _mesh_and_put()
    in_specs = (PartitionSpec("core"),) * (n_params + n_outs)
    out_specs = (PartitionSpec("core"),) * n_outs
    # no donation: the kernel writes every output element, so the zero
    # operands are never read and one staged buffer serves all reps
    sharded = jax.jit(
        shard_map(_body, mesh=mesh, in_specs=in_specs, out_specs=out_specs,
                  check_rep=False),
        keep_unused=True,
    )

    t = time.time()
    staged = staged or {}
    dev_in = [staged.get(name)
              if staged.get(name) is not None
              else put_global([in_maps[c][name] for c in range(n_cores)])
              for name in in_names]
    dev_zero = [staged["__zeros"][i]
                if "__zeros" in staged and i < len(staged["__zeros"])
                else put_global([z] * n_cores)
                for i, z in enumerate(zero_outs)]
    t = _tic("put_dispatch", t)
    compiled = sharded.lower(*dev_in, *dev_zero).compile()
    t = _tic("lower_compile", t)
    for a in dev_in + dev_zero:
        a.block_until_ready()
    t = _tic("transfer_wait", t)

    # warmup: loads the NEFF onto the cores
    warm = compiled(*dev_in, *dev_zero)
    jax.block_until_ready(warm)
    t = _tic("warmup", t)

    # timed region: dispatch + device execute + completion sync.  The
    # output download is NOT in the loop -- its tunnel-bandwidth jitter
    # (tens of ms on 2.5 MB) would swamp the differential timing; the
    # dispatch+sync path is tight (+-1 ms).  Outputs are fetched once
    # below for the returned results.
    tick_idx = out_names.index("tick") if "tick" in out_names else None
    exec_s = float("inf")
    out_arrs = None
    for _ in range(iters):
        t0 = time.time()
        out_arrs = compiled(*dev_in, *dev_zero)
        if tick_idx is not None:
            np.asarray(out_arrs[tick_idx])   # 256 B: forces completion
        else:
            for o in out_arrs:
                o.block_until_ready()
        exec_s = min(exec_s, time.time() - t0)
    for o in out_arrs:
        try:
            o.copy_to_host_async()
        except Exception:
            pass
    host = [np.asarray(o) for o in out_arrs]
    _T["exec_sync"] = exec_s

    results = [
        {name: host[i].reshape(n_cores, *out_avals[i].shape)[c]
         for i, name in enumerate(out_names)}
        for c in range(n_cores)
    ]
    return results, exec_s


def kernel(node_features, edge_features, src, dst,
           ew0, eb0, ew1, eb1, ew2, eb2,
           nw0, nb0, nw1, nb1, nw2, nb2):
    t = time.time()
    node_features = np.asarray(node_features, np.float32)
    edge_features = np.asarray(edge_features, np.float32)
    src = np.asarray(src).astype(np.int64)
    dst = np.asarray(dst).astype(np.int64)
    n_nodes, n_edges = node_features.shape[0], edge_features.shape[0]

    core_nodes, degs, dpos = _partition_graph(src, dst, n_nodes)
    classes = _degree_classes(dpos)
    seg_start = np.concatenate([[0], np.cumsum(dpos)]).astype(np.int64)
    need = int(seg_start[SH])
    cap = ((need + BLK - 1) // BLK) * BLK

    # host-side per-core edge layout (vectorized)
    indeg = np.bincount(dst, minlength=n_nodes)
    perm = np.argsort(dst, kind="stable")          # edges grouped by dst
    estart = np.zeros(n_nodes + 1, np.int64)
    estart[1:] = np.cumsum(indeg)

    cn_arr = np.asarray(core_nodes)                # [CORES, SH]
    j_idx = np.arange(SH)
    slots = ((128 * np.arange(CORES)[:, None] + (j_idx % 128)[None, :])
             * (SHP // 128) + (j_idx // 128)[None, :])
    node_slot = np.empty(n_nodes, np.int64)
    node_slot[cn_arr] = slots
    zero_slot = _slot(0, SH)  # first pad slot of core 0; always exactly 0

    def wrap16(ids):
        a = np.zeros((16, cap // 16), np.int16)
        a[np.arange(cap) % 16, np.arange(cap) // 16] = ids.astype(np.int16)
        return a  # replicated into the 8 Q7 groups on device

    # ---- packed fp16 weights: 9 blocks per layer + bias block ----
    wpk = np.zeros((128, WBLKS * 128), np.float16)
    srcs = [(ew0, 0), (ew0, 1), (ew0, 2), (ew1, 0), (ew2, 0),
            (nw0, 0), (nw0, 1), (nw1, 0), (nw2, 0)]
    for l in range(P):
        for i, (wt, c) in enumerate(srcs):
            wpk[:, (l * 9 + i) * 128:(l * 9 + i + 1) * 128] = \
                np.asarray(wt[l][c * 128:(c + 1) * 128, :], np.float32)
    bofs = (WBLKS - 1) * 128
    for i, bt in enumerate([eb0, eb1, eb2, nb0, nb1, nb2]):
        wpk[:, bofs + 15 * i:bofs + 15 * (i + 1)] = \
            np.asarray(bt, np.float32).T

    # per-core arrays are device_put as soon as they are built, so the
    # tunnel transfer overlaps the remaining prep AND the later BIR build
    # + NEFF compile
    mesh, ns, devices, put_global = _mesh_and_put()
    ef16 = edge_features.astype(np.float16)
    in_maps = []
    shard_bufs = {n: [] for n in ["ef", "nf", "sidxs", "cnt", "wpk"]}
    for k in range(CORES):
        cn = cn_arr[k]
        lens = indeg[cn]
        tot = int(lens.sum())
        rep_j = np.repeat(j_idx, lens)
        within = np.arange(tot) - np.repeat(np.cumsum(lens) - lens, lens)
        cols = seg_start[rep_j] + within
        eids = perm[np.repeat(estart[cn], lens) + within]
        sids = np.full(cap, zero_slot, np.int64)
        sids[cols] = node_slot[src[eids]]
        # pad-edge count per node slot (phantom-edge correction)
        cntk = np.zeros((128, SHP), np.float16)
        cntk[:, :SH] = (dpos - lens)[None, :].astype(np.float16)
        ef = np.zeros((D, cap), np.float16)
        ef[:, cols] = ef16[eids].T
        nf = np.zeros((D, SHP), np.float32)
        nf[:, :SH] = node_features[cn].T
        im = {
            "ef": ef, "nf": nf, "sidxs": wrap16(sids), "cnt": cntk,
            "wpk": wpk[:, k * WPC * 128:(k + 1) * WPC * 128].copy(),
        }
        in_maps.append(im)
        for n in shard_bufs:
            shard_bufs[n].append(jax.device_put(im[n], devices[k]))
    t = _tic("host_prep", t)

    from jax import make_array_from_single_device_arrays as _mk
    staged = {n: _mk((CORES * s[0].shape[0], *s[0].shape[1:]), ns, s)
              for n, s in shard_bufs.items()}
    staged["__zeros"] = [put_global([np.zeros((D, SH), np.float16)] * CORES)]
    t = _tic("stage_put", t)

    _ISA_WARM.join()
    t = _tic("isa_join", t)
    nc = _build_kernel(cap, classes, seg_start)
    t = _tic("build_bir", t)

    _CACHE.update(staged=staged, in_maps=in_maps, cap=cap, classes=classes,
                  seg_start=seg_start)
    results, exec_s = _run_spmd_timed(nc, in_maps, staged=staged)
    global LAST_EXEC_S
    LAST_EXEC_S = exec_s

    t = time.time()
    out = np.empty((n_nodes, D), np.float32)
    for k in range(CORES):
        out[core_nodes[k]] = results[k]["outn"][:, :SH].T.astype(np.float32)
    t = _tic("unshard", t)
    print("[kernel stages] " + "  ".join(f"{k}={v:.3f}s"
                                         for k, v in _T.items()),
          file=sys.stderr, flush=True)
    return out



# revision 49
# speedup vs baseline: 1.0152x; 1.0152x over previous
"""MeshGraphNet processor on 8 Trainium2 NeuronCores.

Strategy (edge-cut graph partition):
  - Nodes dealt round-robin by in-degree rank to 8 cores (1250 each, padded
    to 1280 slots/core).  Each core owns all edges whose dst is local, so
    the segment-sum is core-local.  Per-rank-position degrees are padded to
    the max across cores so one SPMD program (fixed APs) serves every core.
  - Per layer: the local node shard is transposed, cast bf16, and
    AllGather'd into a row-major DRAM table (the collective is fully hidden
    behind compute on HW).  Only n[src] is gathered -- dma_gather transpose
    mode, 512-idx calls spread over 4 SWDGE queues (Q7 descriptor
    generation is the dominant cost and parallelizes across queues).
    n[dst] needs no gather: with edges sorted by dst it is piecewise
    constant, so the per-node dst term ydst = ew0d.T @ n is broadcast to
    edge columns with zero-stride APs and injected into PSUM through an
    identity matmul.  Pad edges would pick up the node's dst term, so a
    per-node "phantom edge" (src=0 input) is run through the same edge MLP
    and (pad count) * phantom is subtracted from the segment sum.
  - Edge MLP runs fp16 end to end (fp16 residual carrier, fp16 h1/h2, raw
    fp16 weights) in a 3-stage software pipeline over 1024-col blocks so
    the PE never stalls on the ACT relus; matmul outputs split per 512-col
    PSUM bank.  Segment-sum is strided DVE reduces by degree class (fp16
    agg); the node MLP keeps an fp32 residual with an fp16 matmul shadow.
  - Wire diet: edge features, weights, and the output ship fp16; weights
    upload 1/8 per core and AllGather on device.  The runner stages inputs
    with per-device puts overlapped with the AOT NEFF compile; the timed
    loop measures dispatch+execute+sync (output download excluded -- its
    tunnel jitter would swamp the differential timing used by test.py).
"""

import sys
import threading
import time
import numpy as np

import jax
import concourse.tile as tile
from concourse import bacc, mybir
from concourse.masks import make_identity


def _warm_isa():
    # get_isa is functools.cache'd: the expensive cffi/pycparser ISA parse
    # (~1.4s) can run while the caller loads inputs and host prep proceeds
    from concourse.isa import get_isa
    get_isa("TRN2")


_ISA_WARM = threading.Thread(target=_warm_isa, daemon=True)
_ISA_WARM.start()

P = 15
D = 128
CORES = 8
SH = 1250          # real nodes per core
SHP = 1280         # padded slots per core (multiple of 128)
NT = CORES * SHP   # 10240 table slots
BLK = 1024         # edge MLP block (2 PSUM banks)
PSP_BUFS = 3       # PSUM pool depth (x2 banks each)
BP_BUFS = 4        # h1/h2 pool depth
GP_BUFS = 14       # gather pool depth
LAST_EXEC_S = None # wall time of the device dispatch+run, set per call
GCH = 512          # edges per dma_gather call
NSWQ = 4           # SWDGE queues (1-4)
QSPREAD = True     # spread gather calls across SWDGE queues
FILL_POOL = False   # all ydst_e fill classes on gpsimd (else alternate)

# packed weight layout: 9 fp16 [128,128] blocks per layer
#   0=ew0s 1=ew0d 2=ew0e 3=ew1 4=ew2 5=nw0a 6=nw0b 7=nw1 8=nw2
# block 135 = biases: eb0@0 eb1@15 eb2@30 nb0@45 nb1@60 nb2@75 (cols, 15 each)
# the pack is sharded 1/8 per core on the wire and AllGather'd on device:
# core c uploads blocks [c*WPC, (c+1)*WPC)
WBLKS = 136
WPC = WBLKS // CORES

F32 = mybir.dt.float32
F16 = mybir.dt.float16
BF16 = mybir.dt.bfloat16
I16 = mybir.dt.int16
AF = mybir.ActivationFunctionType

_T = {}
_CACHE = {}


def rerun_reps(reps, iters=10, ablate=()):
    """Re-run the staged problem with the whole network unrolled `reps`
    times inside one NEFF (same I/O).  Used by test.py for differential
    device timing: (T_reps - T_1)/(reps-1) cancels the tunnel RTT and
    the output-download time, leaving pure device execution per pass.
    Requires a prior kernel() call in this process (uses its staged
    device inputs)."""
    c = _CACHE
    nc = _build_kernel(c["cap"], c["classes"], c["seg_start"], reps=reps,
                       ablate=ablate)
    _, exec_s = _run_spmd_timed(nc, c["in_maps"], staged=c["staged"],
                                iters=iters)
    return exec_s


def _tic(name, t0):
    t = time.time()
    _T[name] = _T.get(name, 0.0) + (t - t0)
    return t


def _slot(core, local):
    # slot id chosen so that byte offset in the AllGather output equals
    # slot*256: out row = 128*core + (local % 128), col block = local // 128
    c, p = divmod(int(local), 128)
    return (128 * int(core) + p) * (SHP // 128) + c


def _partition_graph(src, dst, n_nodes):
    indeg = np.bincount(dst, minlength=n_nodes)
    order = np.argsort(-indeg, kind="stable")  # degree desc
    core_nodes = [[] for _ in range(CORES)]
    for r, v in enumerate(order):
        core_nodes[r % CORES].append(v)
    # local order: degree ascending (reverse of dealt order)
    core_nodes = [list(reversed(cn)) for cn in core_nodes]
    assert all(len(cn) == SH for cn in core_nodes)
    # padded degree per rank position (same for every core)
    degs = np.stack([indeg[cn] for cn in core_nodes])  # [CORES, SH]
    dpos = degs.max(axis=0)                            # [SH] ascending-ish
    return core_nodes, degs, dpos


def _degree_classes(dpos):
    """Contiguous position ranges with equal padded degree (skip d==0)."""
    classes = []
    a = 0
    while a < SH:
        b = a
        while b < SH and dpos[b] == dpos[a]:
            b += 1
        if dpos[a] > 0:
            classes.append((int(dpos[a]), a, b))
        a = b
    return classes


def _build_kernel(cap, classes, seg_start, reps=1, ablate=()):
    nc = bacc.Bacc("TRN2", target_bir_lowering=False, debug=False,
                   num_devices=CORES, num_swdge_queues=NSWQ)
    t_ef = nc.dram_tensor("ef", [D, cap], F16, kind="ExternalInput")
    t_nf = nc.dram_tensor("nf", [D, SHP], F32, kind="ExternalInput")
    t_sxs = nc.dram_tensor("sidxs", [16, cap // 16], I16,
                           kind="ExternalInput")
    t_cnt = nc.dram_tensor("cnt", [128, SHP], F16, kind="ExternalInput")
    t_wpk = nc.dram_tensor("wpk", [128, WPC * 128], F16,
                           kind="ExternalInput")
    t_out = nc.dram_tensor("outn", [D, SH], F16, kind="ExternalOutput")
    # tiny completion beacon: written after the final store, fetched inside
    # the timed loop (a full-output download there would add tunnel jitter;
    # block_until_ready alone acks at dispatch, not completion)
    t_tick = nc.dram_tensor("tick", [1, 16], F16, kind="ExternalOutput")

    nblk = cap // BLK

    with tile.TileContext(nc) as tc:
        with (
            tc.tile_pool(name="persist", bufs=1) as pp,
            tc.tile_pool(name="wts", bufs=2) as wp,
            tc.tile_pool(name="nsh", bufs=2) as np_,
            tc.tile_pool(name="aggp", bufs=2) as ap_,
            tc.tile_pool(name="gath", bufs=GP_BUFS) as gp,
            tc.tile_pool(name="blk", bufs=BP_BUFS) as bp,
            tc.tile_pool(name="stg", bufs=2) as sp,
            tc.tile_pool(name="dgath", bufs=1) as dgp,
            tc.tile_pool(name="ps", bufs=PSP_BUFS, space="PSUM") as psp,
            tc.tile_pool(name="pst", bufs=2, space="PSUM") as pstp,
            tc.tile_pool(name="dram", bufs=2, space="DRAM") as dp,
        ):
            ident = pp.tile([128, 128], F32)
            make_identity(nc, ident[:])
            identF = pp.tile([128, 128], F16, name="identF")
            nc.scalar.activation(identF[:], ident[:], AF.Copy)
            # gather indices ship once (16 partitions) and are replicated
            # into the 8 Q7 groups on device
            sidxs = pp.tile([128, cap // 16], I16, name="sidxs")
            for g in range(8):
                nc.sync.dma_start(sidxs[16 * g:16 * (g + 1), :], t_sxs.ap())
            # per-node pad-edge counts (phantom-edge correction)
            cnt = pp.tile([128, SHP], F16, name="cnt")
            nc.sync.dma_start(cnt[:], t_cnt.ap())
            # phantom pad-edge feature carrier, reset each pass
            pv = pp.tile([D, SHP], F16, name="pv")

            # ---- weight pack arrives 1/8 per core; AllGather on device
            #      (same SBUF->DRAM staging idiom as the node table) ----
            wstg = pp.tile([128, WPC * 128], F16, name="wstg")
            nc.sync.dma_start(wstg[:], t_wpk.ap())
            ag_win = dp.tile([128, WPC * 128], F16, tag="agwi")
            nc.sync.dma_start(ag_win[:], wstg[:])
            ag_wout = dp.tile([128 * CORES, WPC * 128], F16, tag="agwo",
                              addr_space="Shared")
            nc.gpsimd.collective_compute(
                "AllGather", mybir.AluOpType.bypass,
                replica_groups=[list(range(CORES))],
                ins=[ag_win[:].opt()], outs=[ag_wout[:].opt()])

            def wblk(b):
                c, i = divmod(b, WPC)
                return ag_wout[128 * c:128 * (c + 1),
                               128 * i:128 * (i + 1)]

            bc, bi = divmod(WBLKS - 1, WPC)
            b16 = pp.tile([128, 96], F16, name="b16")
            nc.sync.dma_start(
                b16[:, 0:96],
                ag_wout[128 * bc:128 * (bc + 1), 128 * bi:128 * bi + 96])
            bias = pp.tile([128, 96], F32, name="bias")
            nc.scalar.activation(bias[:], b16[:], AF.Copy)
            BOF = dict(eb0=0, eb1=15, eb2=30, nb0=45, nb1=60, nb2=75)

            def bslc(k, l):
                return bias[:, BOF[k] + l:BOF[k] + l + 1]


            # fp16 edge-feature carrier: loaded straight from the wire
            # buffer, residuals accumulate in fp16 (10-bit mantissa)
            e = pp.tile([D, cap], F16, name="e")
            for rep in range(reps):
              # ---- per-pass reload: node shard + edge features ----
              n_cur = np_.tile([D, SHP], F32, tag="n")
              nc.sync.dma_start(n_cur[:], t_nf.ap())
              nc.sync.dma_start(e[:], t_ef.ap())
              nc.gpsimd.memset(pv[:], 0.0)

              for l in range(P):
                # ---- this layer's weights: 9 fp16 block DMAs; only the
                #      src/dst gather blocks (bf16 to match the table) and
                #      nw0a (f32r against the fp32 n_cur) need conversion;
                #      everything else is used as raw fp16 ----
                w16 = wp.tile([128, 9 * 128], F16, tag="w16")
                for i in range(9):
                    nc.sync.dma_start(w16[:, 128 * i:128 * (i + 1)],
                                      wblk(l * 9 + i))
                wsd = wp.tile([128, 128], BF16, tag="wsd")
                nc.scalar.activation(wsd[:], w16[:, 0:128], AF.Copy)
                # fp16 shadow of the node features for the node-MLP matmul
                # (the fp32 carrier stays for the residual)
                n16 = wp.tile([D, SHP], F16, tag="n16")
                nc.scalar.activation(n16[:], n_cur[:], AF.Copy)
                # dst contribution per NODE: ydst = ew0d.T @ n16.  dst-sorted
                # edges make n[dst] piecewise-constant, so the per-edge dst
                # term is a zero-stride broadcast of ydst -- no gather.
                ydst = wp.tile([D, SHP], F16, tag="ydst")
                for (s0, w_) in ((0, 512), (512, 512), (1024, 256)):
                    psd = psp.tile([128, BLK], F32, tag="ps")
                    nc.tensor.matmul(psd[:, :w_], w16[:, 128:256],
                                     n16[:, s0:s0 + w_], start=True,
                                     stop=True)
                    nc.scalar.activation(ydst[:, s0:s0 + w_], psd[:, :w_],
                                         AF.Copy)

                # ---- publish n_cur: transpose shard to rows, cast bf16,
                #      AllGather into the row-major node table ----
                stage = sp.tile([128, SHP], BF16, tag="stage")
                for c in range(SHP // 128):
                    pt = pstp.tile([128, 128], F32, tag="pt")
                    nc.tensor.transpose(
                        pt[:], n_cur[:, c * 128:(c + 1) * 128], ident[:])
                    nc.scalar.activation(
                        stage[:, c * 128:(c + 1) * 128], pt[:], AF.Copy)
                ag_in = dp.tile([128, SHP], BF16, tag="agin")
                nc.sync.dma_start(ag_in[:], stage[:])
                ag_out = dp.tile([128 * CORES, SHP], BF16, tag="agout",
                                 addr_space="Shared")
                # expand ydst to edge columns (broadcast per degree
                # class); pad columns get the node's value too -- corrected
                # via the phantom-edge pass below
                ydst_e = dgp.tile([128, cap], F16, tag="ydste")
                need = int(seg_start[SH])
                if cap > need:
                    nc.gpsimd.memset(ydst_e[:, need:cap], 0.0)
                for ci, (d, a, b) in enumerate(classes):
                    s = seg_start[a]
                    dst_ap = ydst[:, a:b].rearrange(
                        "p (n o) -> p n o", o=1).broadcast_to((128, b - a, d))
                    out_ap = ydst_e[:, s:s + (b - a) * d].rearrange(
                        "p (n d) -> p n d", d=d)
                    eng = nc.gpsimd if (FILL_POOL or ci % 2 == 0) else nc.vector
                    eng.tensor_copy(out_ap, dst_ap)
                if "nocoll" in ablate:
                    nc.sync.dma_start(ag_out[0:128, :], ag_in[:])
                else:
                    nc.gpsimd.collective_compute(
                        "AllGather", mybir.AluOpType.bypass,
                        replica_groups=[list(range(CORES))],
                        ins=[ag_in[:].opt()], outs=[ag_out[:].opt()])
                table_ap = ag_out[:].rearrange("r (c f) -> (r c) f", f=D)

                # ---- edge MLP blocks, 3-stage software pipeline: the PE
                #      queue sees mm1(j), mm2(j-1), mm3(j-2) per iteration
                #      so it never stalls on the ACT relu of its own block
                # a matmul output must stay inside one 512-col PSUM bank:
                # emit per-bank halves (ACT/DVE still see 1024-wide tiles)
                def mm(ps, lhsT, rhs, start, stop):
                    for h in range(BLK // 512):
                        hs = slice(h * 512, (h + 1) * 512)
                        nc.tensor.matmul(ps[:, hs], lhsT, rhs[:, hs],
                                         start=start, stop=stop)

                st_h1 = {}
                st_h2 = {}
                for j in range(nblk + 2):
                    if j < nblk:
                        gs = gp.tile([128, 1, BLK], BF16, tag="gs")
                        for k in range(BLK // GCH):
                            if "srcg_half" in ablate and k % 2 == 1:
                                continue
                            g = j * (BLK // GCH) + k
                            isl = slice(g * (GCH // 16),
                                        (g + 1) * (GCH // 16))
                            nc.gpsimd.dma_gather(
                                gs[:, :, k * GCH:(k + 1) * GCH], table_ap,
                                sidxs[:, isl], GCH, GCH, D, transpose=True,
                                queue_num=(g % NSWQ) if QSPREAD else 0,
                            )
                        eblk = e[:, j * BLK:(j + 1) * BLK]
                        ps1 = psp.tile([128, BLK], F32, tag="ps")
                        mm(ps1, wsd[:, 0:128], gs[:, 0, :],
                           start=True, stop=False)
                        mm(ps1, identF[:],
                           ydst_e[:, j * BLK:(j + 1) * BLK],
                           start=False, stop=False)
                        mm(ps1, w16[:, 256:384], eblk,
                           start=False, stop=True)
                        h1 = bp.tile([D, BLK], F16, tag="h1")
                        nc.scalar.activation(h1[:], ps1[:], AF.Relu,
                                             bias=bslc("eb0", l))
                        st_h1[j] = h1
                    if j >= 1 and j - 1 < nblk:
                        ps2 = psp.tile([128, BLK], F32, tag="ps")
                        mm(ps2, w16[:, 384:512], st_h1.pop(j - 1)[:],
                           start=True, stop=True)
                        h2 = bp.tile([D, BLK], F16, tag="h2")
                        nc.scalar.activation(h2[:], ps2[:], AF.Relu,
                                             bias=bslc("eb1", l))
                        st_h2[j - 1] = h2
                    if j >= 2:
                        jj = j - 2
                        ps3 = psp.tile([128, BLK], F32, tag="ps")
                        mm(ps3, w16[:, 512:640], st_h2.pop(jj)[:],
                           start=True, stop=True)
                        ebj = e[:, jj * BLK:(jj + 1) * BLK]
                        # e += ps3 + eb2, fused (DVE: gpsimd can't read PSUM)
                        nc.vector.scalar_tensor_tensor(
                            out=ebj, in0=ps3[:], scalar=bslc("eb2", l),
                            in1=ebj, op0=mybir.AluOpType.add,
                            op1=mybir.AluOpType.add)

                # ---- phantom pad-edge pass: pad columns run the edge
                # MLP with src=0 and the node's own dst term, exactly like
                # the real pads embedded in e, so (pad count) * pv can be
                # subtracted from the segment sum afterwards ----
                for (s0, w_) in ((0, 512), (512, 512), (1024, 256)):
                    psA = psp.tile([128, BLK], F32, tag="ps")
                    nc.tensor.matmul(psA[:, :w_], identF[:],
                                     ydst[:, s0:s0 + w_], start=True,
                                     stop=False)
                    nc.tensor.matmul(psA[:, :w_], w16[:, 256:384],
                                     pv[:, s0:s0 + w_], start=False,
                                     stop=True)
                    h1p = bp.tile([D, BLK], F16, tag="h1")
                    nc.scalar.activation(h1p[:, :w_], psA[:, :w_], AF.Relu,
                                         bias=bslc("eb0", l))
                    psB = psp.tile([128, BLK], F32, tag="ps")
                    nc.tensor.matmul(psB[:, :w_], w16[:, 384:512],
                                     h1p[:, :w_], start=True, stop=True)
                    h2p = bp.tile([D, BLK], F16, tag="h2")
                    nc.scalar.activation(h2p[:, :w_], psB[:, :w_], AF.Relu,
                                         bias=bslc("eb1", l))
                    psC = psp.tile([128, BLK], F32, tag="ps")
                    nc.tensor.matmul(psC[:, :w_], w16[:, 512:640],
                                     h2p[:, :w_], start=True, stop=True)
                    nc.vector.scalar_tensor_tensor(
                        out=pv[:, s0:s0 + w_], in0=psC[:, :w_],
                        scalar=bslc("eb2", l), in1=pv[:, s0:s0 + w_],
                        op0=mybir.AluOpType.add, op1=mybir.AluOpType.add)

                # ---- segment sum (edges sorted by dst, degree classes);
                #      fp16 in/out rides the DVE 2x 16-bit mode ----
                agg = ap_.tile([D, SHP], F16, tag="agg")
                nc.gpsimd.memset(agg[:], 0.0)
                with nc.allow_low_precision("fp16 agg, values O(10)"):
                    for (d, a, b) in classes:
                        s = seg_start[a]
                        seg = e[:, s:s + (b - a) * d].rearrange(
                            "p (n d) -> p n d", d=d)
                        nc.vector.tensor_reduce(
                            agg[:, a:b], seg, axis=mybir.AxisListType.X,
                            op=mybir.AluOpType.add)
                    # subtract the pad-edge pollution: agg -= cnt * pv
                    tmpc = sp.tile([D, SHP], F16, tag="tmpc")
                    nc.vector.tensor_tensor(tmpc[:], pv[:], cnt[:],
                                            op=mybir.AluOpType.mult)
                    nc.vector.tensor_tensor(agg[:], agg[:], tmpc[:],
                                            op=mybir.AluOpType.subtract)

                # ---- node MLP on local shard ----
                n_new = np_.tile([D, SHP], F32, tag="n")
                for (s0, w_) in ((0, 512), (512, 512), (1024, 256)):
                    ps1 = psp.tile([128, BLK], F32, tag="ps")
                    nc.tensor.matmul(ps1[:, :w_], w16[:, 640:768],
                                     n16[:, s0:s0 + w_], start=True,
                                     stop=False)
                    nc.tensor.matmul(ps1[:, :w_], w16[:, 768:896],
                                     agg[:, s0:s0 + w_], start=False,
                                     stop=True)
                    h1 = bp.tile([D, BLK], F16, tag="h1")
                    nc.scalar.activation(h1[:, :w_], ps1[:, :w_], AF.Relu,
                                         bias=bslc("nb0", l))
                    ps2 = psp.tile([128, BLK], F32, tag="ps")
                    nc.tensor.matmul(ps2[:, :w_], w16[:, 896:1024],
                                     h1[:, :w_], start=True, stop=True)
                    h2 = bp.tile([D, BLK], F16, tag="h2")
                    nc.scalar.activation(h2[:, :w_], ps2[:, :w_], AF.Relu,
                                         bias=bslc("nb1", l))
                    ps3 = psp.tile([128, BLK], F32, tag="ps")
                    nc.tensor.matmul(ps3[:, :w_], w16[:, 1024:1152],
                                     h2[:, :w_], start=True, stop=True)
                    nc.vector.scalar_tensor_tensor(
                        out=n_new[:, s0:s0 + w_], in0=ps3[:, :w_],
                        scalar=bslc("nb2", l), in1=n_cur[:, s0:s0 + w_],
                        op0=mybir.AluOpType.add, op1=mybir.AluOpType.add)

                if l == P - 1:
                    o16 = sp.tile([D, SHP], F16, tag="o16")
                    nc.scalar.activation(o16[:, :SH], n_new[:, :SH], AF.Copy)
                    nc.sync.dma_start(t_out.ap(), o16[:, :SH])
                    nc.sync.dma_start(t_tick.ap(), o16[0:1, 0:16])
                n_cur = n_new

    nc.compile()
    return nc


def _mesh_and_put():
    """(mesh, sharding, devices, put_global) for the 8-core mesh."""
    from jax.sharding import Mesh, PartitionSpec, NamedSharding

    devices = jax.devices()[:CORES]
    mesh = Mesh(np.asarray(devices), ("core",))
    ns = NamedSharding(mesh, PartitionSpec("core"))

    def put_global(arrs):
        shards = [jax.device_put(np.asarray(arrs[c]), devices[c])
                  for c in range(CORES)]
        shape = (CORES * shards[0].shape[0], *shards[0].shape[1:])
        return jax.make_array_from_single_device_arrays(shape, ns, shards)

    return mesh, ns, devices, put_global


def _run_spmd_timed(nc, in_maps, staged=None, iters=10):
    """run_bass_kernel_spmd replacement: async device_put (pre-staged where
    possible) overlapped with AOT NEFF compile; only
    dispatch+execute+output-fetch is timed."""
    from jax.sharding import PartitionSpec
    from jax.experimental.shard_map import shard_map
    from concourse.bass2jax import (_bass_exec_p, install_neuronx_cc_hook,
                                    partition_id_tensor)

    install_neuronx_cc_hook()
    n_cores = len(in_maps)
    partition_name = (nc.partition_id_tensor.name
                      if nc.partition_id_tensor else None)

    in_names, out_names, out_avals, zero_outs = [], [], [], []
    for alloc in nc.m.functions[0].allocations:
        if not isinstance(alloc, mybir.MemoryLocationSet):
            continue
        name = alloc.memorylocations[0].name
        if alloc.kind == "ExternalInput":
            if name != partition_name:
                in_names.append(name)
        elif alloc.kind == "ExternalOutput":
            shape = tuple(alloc.tensor_shape)
            dtype = mybir.dt.np(alloc.dtype)
            out_names.append(name)
            out_avals.append(jax.core.ShapedArray(shape, dtype))
            zero_outs.append(np.zeros(shape, dtype))
    n_params = len(in_names)
    n_outs = len(out_avals)
    all_in_names = list(in_names) + list(out_names)
    if partition_name is not None:
        all_in_names.append(partition_name)

    def _body(*args):
        operands = list(args)
        if partition_name is not None:
            operands.append(partition_id_tensor())
        outs = _bass_exec_p.bind(
            *operands,
            out_avals=tuple(out_avals),
            in_names=tuple(all_in_names),
            out_names=tuple(out_names),
            lowering_input_output_aliases=(),
            sim_require_finite=True,
            sim_require_nnan=True,
            nc=nc,
        )
        return tuple(outs)

    mesh, ns, _, put_global = _mesh_and_put()
    in_specs = (PartitionSpec("core"),) * (n_params + n_outs)
    out_specs = (PartitionSpec("core"),) * n_outs
    # no donation: the kernel writes every output element, so the zero
    # operands are never read and one staged buffer serves all reps
    sharded = jax.jit(
        shard_map(_body, mesh=mesh, in_specs=in_specs, out_specs=out_specs,
                  check_rep=False),
        keep_unused=True,
    )

    t = time.time()
    staged = staged or {}
    dev_in = [staged.get(name)
              if staged.get(name) is not None
              else put_global([in_maps[c][name] for c in range(n_cores)])
              for name in in_names]
    dev_zero = [staged["__zeros"][i]
                if "__zeros" in staged and i < len(staged["__zeros"])
                else put_global([z] * n_cores)
                for i, z in enumerate(zero_outs)]
    t = _tic("put_dispatch", t)
    compiled = sharded.lower(*dev_in, *dev_zero).compile()
    t = _tic("lower_compile", t)
    for a in dev_in + dev_zero:
        a.block_until_ready()
    t = _tic("transfer_wait", t)

    # warmup: loads the NEFF onto the cores
    warm = compiled(*dev_in, *dev_zero)
    jax.block_until_ready(warm)
    t = _tic("warmup", t)

    # timed region: dispatch + device execute + completion sync.  The
    # output download is NOT in the loop -- its tunnel-bandwidth jitter
    # (tens of ms on 2.5 MB) would swamp the differential timing; the
    # dispatch+sync path is tight (+-1 ms).  Outputs are fetched once
    # below for the returned results.
    tick_idx = out_names.index("tick") if "tick" in out_names else None
    exec_s = float("inf")
    out_arrs = None
    for _ in range(iters):
        t0 = time.time()
        out_arrs = compiled(*dev_in, *dev_zero)
        if tick_idx is not None:
            np.asarray(out_arrs[tick_idx])   # 256 B: forces completion
        else:
            for o in out_arrs:
                o.block_until_ready()
        exec_s = min(exec_s, time.time() - t0)
    for o in out_arrs:
        try:
            o.copy_to_host_async()
        except Exception:
            pass
    host = [np.asarray(o) for o in out_arrs]
    _T["exec_sync"] = exec_s

    results = [
        {name: host[i].reshape(n_cores, *out_avals[i].shape)[c]
         for i, name in enumerate(out_names)}
        for c in range(n_cores)
    ]
    return results, exec_s


def kernel(node_features, edge_features, src, dst,
           ew0, eb0, ew1, eb1, ew2, eb2,
           nw0, nb0, nw1, nb1, nw2, nb2):
    t = time.time()
    node_features = np.asarray(node_features, np.float32)
    edge_features = np.asarray(edge_features, np.float32)
    src = np.asarray(src).astype(np.int64)
    dst = np.asarray(dst).astype(np.int64)
    n_nodes, n_edges = node_features.shape[0], edge_features.shape[0]

    core_nodes, degs, dpos = _partition_graph(src, dst, n_nodes)
    classes = _degree_classes(dpos)
    seg_start = np.concatenate([[0], np.cumsum(dpos)]).astype(np.int64)
    need = int(seg_start[SH])
    cap = ((need + BLK - 1) // BLK) * BLK

    # host-side per-core edge layout (vectorized)
    indeg = np.bincount(dst, minlength=n_nodes)
    perm = np.argsort(dst, kind="stable")          # edges grouped by dst
    estart = np.zeros(n_nodes + 1, np.int64)
    estart[1:] = np.cumsum(indeg)

    cn_arr = np.asarray(core_nodes)                # [CORES, SH]
    j_idx = np.arange(SH)
    slots = ((128 * np.arange(CORES)[:, None] + (j_idx % 128)[None, :])
             * (SHP // 128) + (j_idx // 128)[None, :])
    node_slot = np.empty(n_nodes, np.int64)
    node_slot[cn_arr] = slots
    zero_slot = _slot(0, SH)  # first pad slot of core 0; always exactly 0

    def wrap16(ids):
        a = np.zeros((16, cap // 16), np.int16)
        a[np.arange(cap) % 16, np.arange(cap) // 16] = ids.astype(np.int16)
        return a  # replicated into the 8 Q7 groups on device

    # ---- packed fp16 weights: 9 blocks per layer + bias block ----
    wpk = np.zeros((128, WBLKS * 128), np.float16)
    srcs = [(ew0, 0), (ew0, 1), (ew0, 2), (ew1, 0), (ew2, 0),
            (nw0, 0), (nw0, 1), (nw1, 0), (nw2, 0)]
    for l in range(P):
        for i, (wt, c) in enumerate(srcs):
            wpk[:, (l * 9 + i) * 128:(l * 9 + i + 1) * 128] = \
                np.asarray(wt[l][c * 128:(c + 1) * 128, :], np.float32)
    bofs = (WBLKS - 1) * 128
    for i, bt in enumerate([eb0, eb1, eb2, nb0, nb1, nb2]):
        wpk[:, bofs + 15 * i:bofs + 15 * (i + 1)] = \
            np.asarray(bt, np.float32).T

    # per-core arrays are device_put as soon as they are built, so the
    # tunnel transfer overlaps the remaining prep AND the later BIR build
    # + NEFF compile
    mesh, ns, devices, put_global = _mesh_and_put()
    ef16 = edge_features.astype(np.float16)
    in_maps = []
    shard_bufs = {n: [] for n in ["ef", "nf", "sidxs", "cnt", "wpk"]}
    for k in range(CORES):
        cn = cn_arr[k]
        lens = indeg[cn]
        tot = int(lens.sum())
        rep_j = np.repeat(j_idx, lens)
        within = np.arange(tot) - np.repeat(np.cumsum(lens) - lens, lens)
        cols = seg_start[rep_j] + within
        eids = perm[np.repeat(estart[cn], lens) + within]
        sids = np.full(cap, zero_slot, np.int64)
        sids[cols] = node_slot[src[eids]]
        # pad-edge count per node slot (phantom-edge correction)
        cntk = np.zeros((128, SHP), np.float16)
        cntk[:, :SH] = (dpos - lens)[None, :].astype(np.float16)
        ef = np.zeros((D, cap), np.float16)
        ef[:, cols] = ef16[eids].T
        nf = np.zeros((D, SHP), np.float32)
        nf[:, :SH] = node_features[cn].T
        im = {
            "ef": ef, "nf": nf, "sidxs": wrap16(sids), "cnt": cntk,
            "wpk": wpk[:, k * WPC * 128:(k + 1) * WPC * 128].copy(),
        }
        in_maps.append(im)
        for n in shard_bufs:
            shard_bufs[n].append(jax.device_put(im[n], devices[k]))
    t = _tic("host_prep", t)

    from jax import make_array_from_single_device_arrays as _mk
    staged = {n: _mk((CORES * s[0].shape[0], *s[0].shape[1:]), ns, s)
              for n, s in shard_bufs.items()}
    staged["__zeros"] = [put_global([np.zeros((D, SH), np.float16)] * CORES)]
    t = _tic("stage_put", t)

    _ISA_WARM.join()
    t = _tic("isa_join", t)
    nc = _build_kernel(cap, classes, seg_start)
    t = _tic("build_bir", t)

    _CACHE.update(staged=staged, in_maps=in_maps, cap=cap, classes=classes,
                  seg_start=seg_start)
    results, exec_s = _run_spmd_timed(nc, in_maps, staged=staged)
    global LAST_EXEC_S
    LAST_EXEC_S = exec_s

    t = time.time()
    out = np.empty((n_nodes, D), np.float32)
    for k in range(CORES):
        out[core_nodes[k]] = results[k]["outn"][:, :SH].T.astype(np.float32)
    t = _tic("unshard", t)
    print("[kernel stages] " + "  ".join(f"{k}={v:.3f}s"
                                         for k, v in _T.items()),
          file=sys.stderr, flush=True)
    return out

